# revision 12
# baseline (speedup 1.0000x reference)
"""DTNNStep Bass kernel for Trainium2 (8 NeuronCores, data-parallel over batch).

Computes, per molecule b:
    dist_h = dist @ W_df + b_df              # [N, N, H]
    atom_h = atom @ W_cf + b_cf              # [N, H]
    gated  = dist_h * atom_h[None, :, :]     # broadcast over i
    out    = tanh((gated @ W_fc) * mask)     # mask == 1 in this benchmark
    result = out.sum(axis=1) + atom          # [N, F]

Pipeline (v2): dist is loaded in its NATURAL [i, (j d)] layout with the
f32->bf16 cast done on the DMA wire (SWDGE), giving 12.8KB contiguous
descriptors instead of the 200B gather descriptors a j-partition rearrange
load produces (which made the SDMA engines the bottleneck).  Per-j PE
transposes build a fully transposed molecule distT[d, (j i)] in SBUF; the
matmuls then consume columns in (i, j) order through strided APs so the
j-reduction is an innermost contiguous bf16 2x reduce.  b_df is folded into
the gate as a fused scalar_tensor_tensor (dist_h + b_df) * atom_h.
"""

import os
import sys

import numpy as np

for _p in ("/opt/trn_rl_repo", os.path.expanduser("~/.axon_site/_ro/trn_rl_repo")):
    if os.path.isdir(_p) and _p not in sys.path:
        sys.path.insert(0, _p)

import concourse.bass as bass
import concourse.tile as tile
from concourse import bacc, mybir
from concourse.bass import ds
from concourse.bass_utils import run_bass_kernel_spmd
from concourse.masks import make_identity

B, N, NF, ND, NH = 16, 128, 64, 100, 64
NCORES = 8
BPC = B // NCORES  # molecules per core

F32 = mybir.dt.float32
BF16 = mybir.dt.bfloat16

JC = 32  # j's per dist DMA chunk
JB = 8  # j's per transpose batch (one PSUM bank)


def _emit(tc):
    nc = tc.nc
    dist = nc.dram_tensor("dist", (BPC, N, N, ND), F32, kind="ExternalInput").ap()
    atom = nc.dram_tensor("atom", (BPC, N, NF), F32, kind="ExternalInput").ap()
    w_cf = nc.dram_tensor("w_cf", (NF, NH), F32, kind="ExternalInput").ap()
    w_df = nc.dram_tensor("w_df", (ND, NH), F32, kind="ExternalInput").ap()
    w_fc = nc.dram_tensor("w_fc", (NH, NF), F32, kind="ExternalInput").ap()
    b_cf = nc.dram_tensor("b_cf", (1, NH), F32, kind="ExternalInput").ap()
    b_df = nc.dram_tensor("b_df", (1, NH), F32, kind="ExternalInput").ap()
    out = nc.dram_tensor("out", (BPC, N, NF), F32, kind="ExternalOutput").ap()

    with (
        tc.tile_pool(name="consts", bufs=1) as consts,
        tc.tile_pool(name="loads", bufs=4) as loads,
        tc.tile_pool(name="big", bufs=2) as big,
        tc.tile_pool(name="work", bufs=4) as work,
        tc.tile_pool(name="perb", bufs=2) as perb,
        tc.tile_pool(name="ppool", bufs=2, space="PSUM") as ppool,
    ):
        identity = consts.tile([128, 128], F32)
        make_identity(nc, identity)
        identity_bf = consts.tile([128, 128], BF16)
        make_identity(nc, identity_bf)
        ones_f32 = consts.tile([1, N], F32)
        nc.vector.memset(ones_f32, 1.0)

        # Preload the tanh table set (~2.7us) while the first DMAs are in flight.
        warm_tanh = consts.tile([1, 8], F32)
        nc.scalar.activation(warm_tanh, ones_f32[:, :8], mybir.ActivationFunctionType.Tanh)

        # W_df in bf16 for mm1 (bias handled in the gate).
        w_df_f = consts.tile([ND, NH], F32)
        nc.sync.dma_start(w_df_f, w_df)
        w_df_bf = consts.tile([ND, NH], BF16)
        nc.vector.tensor_copy(w_df_bf, w_df_f)

        # W_fc stacked twice vertically so the partition-hi mm2 has its
        # stationary at the same base partition as its rhs.
        w_fc_f = consts.tile([2 * NH, NF], F32)
        nc.sync.dma_start(w_fc_f[:NH], w_fc)
        nc.sync.dma_start(w_fc_f[NH:], w_fc)
        w_fc_bf = consts.tile([2 * NH, NF], BF16)
        nc.vector.tensor_copy(w_fc_bf, w_fc_f)

        # W_cf plain; b_cf as a row for the K=1 bias fold.
        w_cf_sb = consts.tile([NF, NH], F32)
        nc.sync.dma_start(w_cf_sb, w_cf)
        b_cf_sb = consts.tile([1, NH], F32)
        nc.sync.dma_start(b_cf_sb, b_cf)

        # b_df as a per-partition column, duplicated for both halves.
        b_df_col_f = consts.tile([2 * NH, 1], F32)
        nc.sync.dma_start(b_df_col_f[:NH], b_df.rearrange("a h -> h a"))
        nc.sync.dma_start(b_df_col_f[NH:], b_df.rearrange("a h -> h a"))

        # --- per-molecule prep.  Partition half u of the main pipeline handles
        # j-quad 8t+4u+j', so atom_hGB[u*NH+h, 4t+j'] = atom_h[8t+4u+j', h].
        atom_hGBs = []
        for b in range(BPC):
            atom_in = loads.tile([N, NF], F32, tag="atom_in")
            nc.sync.dma_start(atom_in, atom[b])
            atomT_ps = ppool.tile([NF, N], F32, tag="out2")
            nc.tensor.transpose(atomT_ps, atom_in, identity)
            atomT = work.tile([NF, N], F32, tag="atomT")
            nc.vector.tensor_copy(atomT, atomT_ps)
            ah_ps = ppool.tile([2 * NH, N // 2], F32, tag="out2")
            atomT_q = atomT.rearrange("f (t j) -> f t j", j=8)
            for u in range(2):
                nc.tensor.matmul(
                    ah_ps[ds(u * NH, NH)],
                    w_cf_sb,
                    atomT_q[:, :, ds(4 * u, 4)],
                    start=True,
                    stop=False,
                )
                nc.tensor.matmul(
                    ah_ps[ds(u * NH, NH)],
                    b_cf_sb,
                    ones_f32[:, : N // 2],
                    start=False,
                    stop=True,
                )
            atom_hGB = perb.tile([2 * NH, N // 2], BF16, tag="atom_hGB")
            nc.vector.tensor_copy(atom_hGB, ah_ps)
            atom_hGBs.append(atom_hGB)

        distTs = [
            big.tile([ND, N * N], BF16, tag="distT", name=f"distT{b}")
            for b in range(BPC)
        ]
        tanh_fulls = [
            big.tile([2 * NF, N * N // 2], BF16, tag="tanh_full", name=f"tanhf{b}")
            for b in range(BPC)
        ]
        th2s = [
            big.tile([2 * NF, N * N // 4], F32, tag="th2", name=f"th2{b}")
            for b in range(BPC)
        ]

        def front_chunk(b, c):
            # load JC j's for all i in natural layout, casting on the wire
            dist_bf = loads.tile([N, JC * ND], BF16, tag="dist_bf")
            nc.gpsimd.dma_start(
                dist_bf, dist[b, :, ds(c * JC, JC), :].rearrange("i j d -> i (j d)")
            )
            for kk in range(JC // JB):
                tp_ps = ppool.tile([ND, JB * N], BF16, tag="tp")
                for q in range(JB):
                    nc.tensor.transpose(
                        tp_ps[:, ds(q * N, N)],
                        dist_bf[:, ds((kk * JB + q) * ND, ND)],
                        identity_bf,
                    )
                j0 = c * JC + kk * JB
                # split the PSUM->SBUF copies across vector and scalar
                dst = distTs[b][:, ds(j0 * N, JB * N)]
                if (c * (JC // JB) + kk) % 2 == 0:
                    nc.vector.tensor_copy(dst, tp_ps)
                else:
                    nc.scalar.copy(dst, tp_ps)

        def main_group_pair(b, tp):
            # two groups (2*tp, 2*tp+1); group t covers j in [8t, 8t+8), with
            # j-quads 8t..8t+3 / 8t+4..8t+7 on the partition halves.
            distT = distTs[b]
            atom_hGB = atom_hGBs[b]
            out2_ps = ppool.tile([2 * NF, 2 * 4 * N], F32, tag="out2")
            for g in range(2):
                t = 2 * tp + g
                out1_ps = ppool.tile([2 * NH, 4 * N], F32, tag="out1")
                for u in range(2):
                    nc.tensor.matmul(
                        out1_ps[ds(u * NH, NH)],
                        w_df_bf,
                        distT[:, ds((2 * t + u) * 4 * N, 4 * N)],
                        start=True,
                        stop=True,
                    )
                gatedT = work.tile([2 * NH, 4 * N], BF16, tag="gatedT")
                nc.vector.scalar_tensor_tensor(
                    gatedT.rearrange("h (j i) -> h j i", j=4),
                    out1_ps.rearrange("h (j i) -> h j i", j=4),
                    b_df_col_f,
                    atom_hGB[:, ds(4 * t, 4), None].to_broadcast((2 * NH, 4, N)),
                    op0=mybir.AluOpType.add,
                    op1=mybir.AluOpType.mult,
                )
                nc.tensor.matmul(
                    out2_ps[:NF, ds(g * 4 * N, 4 * N)],
                    w_fc_bf[:NH],
                    gatedT[:NH],
                    start=True,
                    stop=True,
                )
                nc.tensor.matmul(
                    out2_ps[NF:, ds(g * 4 * N, 4 * N)],
                    w_fc_bf[NH:],
                    gatedT[NH:],
                    start=True,
                    stop=True,
                )
            nc.scalar.activation(
                tanh_fulls[b][:, ds(tp * 8 * N, 8 * N)],
                out2_ps,
                mybir.ActivationFunctionType.Tanh,
            )

        def reduce_mol(b):
            tf = tanh_fulls[b]
            th2 = th2s[b]
            half = N * N // 4
            # stage 1 on gpsimd: fold the two j-halves (SBUF->SBUF)
            nc.vector.tensor_tensor(
                th2, tf[:, :half], tf[:, half:], mybir.AluOpType.add
            )
            # stage 2 on vector: contiguous pairwise-add tree over the 32
            # remaining 128-col blocks
            res = perb.tile([2 * NF, N], F32, tag="res")
            widths = [2048, 1024, 512, 256, 128]
            srcs = th2
            for w in widths:
                dst = res if w == 128 else perb.tile([2 * NF, w], F32, tag=f"tr{w}")
                nc.vector.tensor_add(dst, srcs[:, :w], srcs[:, w : 2 * w])
                srcs = dst
            # finalize: out[b] = res^T (both partition halves summed) + atom[b]
            atom_nat = loads.tile([N, NF], F32, tag="atom_nat")
            nc.sync.dma_start(atom_nat, atom[b])
            acc = None
            for u in range(2):
                resT_ps = ppool.tile([N, NF], F32, tag="out2")
                nc.tensor.transpose(
                    resT_ps,
                    res[ds(u * NF, NF)],
                    identity[ds(u * NF, NF), ds(u * NF, NF)],
                )
                nxt = work.tile([N, NF], F32, tag=f"fin{u}")
                nc.vector.tensor_add(nxt, resT_ps, atom_nat if u == 0 else acc)
                acc = nxt
            nc.sync.dma_start(out[b], acc)

        # molecule 0 front, then molecule 0 mains interleaved with molecule 1
        # front (emission order biases the Tile scheduler's priorities).
        for c in range(N // JC):
            front_chunk(0, c)
        for tp in range(N // 8 // 2):
            if tp % 2 == 0:
                front_chunk(1, tp // 2)
            main_group_pair(0, tp)
        reduce_mol(0)
        for tp in range(N // 8 // 2):
            main_group_pair(1, tp)
        reduce_mol(1)


_NC_CACHE = None


def _get_nc():
    global _NC_CACHE
    if _NC_CACHE is None:
        nc = bacc.Bacc("TRN2", target_bir_lowering=False, debug=False)
        with tile.TileContext(nc) as tc:
            _emit(tc)
        nc.compile()
        _NC_CACHE = nc
    return _NC_CACHE


def _numpy_reference(atom, dist, mask, w_cf, w_df, w_fc, b_cf, b_df):
    dist_h = np.einsum("bijd,dh->bijh", dist, w_df) + b_df
    atom_h = np.einsum("bjf,fh->bjh", atom, w_cf) + b_cf
    gated = dist_h * atom_h[:, None, :, :]
    o = np.einsum("bijh,hf->bijf", gated, w_fc)
    o = np.tanh(o * mask[..., None])
    return (o.sum(axis=2) + atom).astype(np.float32)


def run_sharded(inputs, trace=False):
    """Shard over the batch axis, run on 8 cores, gather. Returns (out, results)."""
    atom = np.ascontiguousarray(np.asarray(inputs["atom_features"], np.float32))
    dist = np.ascontiguousarray(np.asarray(inputs["distance_matrix"], np.float32))
    w_cf = np.ascontiguousarray(np.asarray(inputs["W_cf"], np.float32))
    w_df = np.ascontiguousarray(np.asarray(inputs["W_df"], np.float32))
    w_fc = np.ascontiguousarray(np.asarray(inputs["W_fc"], np.float32))
    b_cf = np.asarray(inputs["b_cf"], np.float32).reshape(1, NH)
    b_df = np.asarray(inputs["b_df"], np.float32).reshape(1, NH)

    nc = _get_nc()
    in_maps = []
    for c in range(NCORES):
        sl = slice(c * BPC, (c + 1) * BPC)
        in_maps.append(
            {
                "dist": dist[sl],
                "atom": atom[sl],
                "w_cf": w_cf,
                "w_df": w_df,
                "w_fc": w_fc,
                "b_cf": b_cf,
                "b_df": b_df,
            }
        )
    res = run_bass_kernel_spmd(nc, in_maps, core_ids=list(range(NCORES)), trace=trace)
    out = np.concatenate([res.results[c]["out"] for c in range(NCORES)], axis=0)
    return out, res


def kernel(**inputs) -> np.ndarray:
    mask = np.asarray(inputs["distance_matrix_mask"], np.float32)
    if not np.all(mask == 1.0):
        # The hardware pipeline folds the (always-ones) mask away; keep a
        # correct path for arbitrary masks.
        return _numpy_reference(
            np.asarray(inputs["atom_features"], np.float32),
            np.asarray(inputs["distance_matrix"], np.float32),
            mask,
            np.asarray(inputs["W_cf"], np.float32),
            np.asarray(inputs["W_df"], np.float32),
            np.asarray(inputs["W_fc"], np.float32),
            np.asarray(inputs["b_cf"], np.float32),
            np.asarray(inputs["b_df"], np.float32),
        )
    out, _ = run_sharded(inputs)
    return out


# revision 13
# speedup vs baseline: 24149.9959x; 24149.9959x over previous
"""DTNNStep Bass kernel for Trainium2 (8 NeuronCores, data-parallel over batch).

Computes, per molecule b:
    dist_h = dist @ W_df + b_df              # [N, N, H]
    atom_h = atom @ W_cf + b_cf              # [N, H]
    gated  = dist_h * atom_h[None, :, :]     # broadcast over i
    out    = tanh((gated @ W_fc) * mask)     # mask == 1 in this benchmark
    result = out.sum(axis=1) + atom          # [N, F]

Pipeline (v2): dist is loaded in its NATURAL [i, (j d)] layout with the
f32->bf16 cast done on the DMA wire (SWDGE), giving 12.8KB contiguous
descriptors instead of the 200B gather descriptors a j-partition rearrange
load produces (which made the SDMA engines the bottleneck).  Per-j PE
transposes build a fully transposed molecule distT[d, (j i)] in SBUF; the
matmuls then consume columns in (i, j) order through strided APs so the
j-reduction is an innermost contiguous bf16 2x reduce.  b_df is folded into
the gate as a fused scalar_tensor_tensor (dist_h + b_df) * atom_h.
"""

import os
import sys

import numpy as np

for _p in ("/opt/trn_rl_repo", os.path.expanduser("~/.axon_site/_ro/trn_rl_repo")):
    if os.path.isdir(_p) and _p not in sys.path:
        sys.path.insert(0, _p)

import concourse.bass as bass
import concourse.tile as tile
from concourse import bacc, mybir
from concourse.bass import ds
from concourse.bass_utils import run_bass_kernel_spmd
from concourse.masks import make_identity

B, N, NF, ND, NH = 16, 128, 64, 100, 64
NCORES = 8
BPC = B // NCORES  # molecules per core

F32 = mybir.dt.float32
BF16 = mybir.dt.bfloat16

JC = 32  # j's per dist DMA chunk
JB = 8  # j's per transpose batch (one PSUM bank)


def _emit(tc):
    nc = tc.nc
    dist = nc.dram_tensor("dist", (BPC, N, N, ND), F32, kind="ExternalInput").ap()
    atom = nc.dram_tensor("atom", (BPC, N, NF), F32, kind="ExternalInput").ap()
    w_cf = nc.dram_tensor("w_cf", (NF, NH), F32, kind="ExternalInput").ap()
    w_df = nc.dram_tensor("w_df", (ND, NH), F32, kind="ExternalInput").ap()
    w_fc = nc.dram_tensor("w_fc", (NH, NF), F32, kind="ExternalInput").ap()
    b_cf = nc.dram_tensor("b_cf", (1, NH), F32, kind="ExternalInput").ap()
    b_df = nc.dram_tensor("b_df", (1, NH), F32, kind="ExternalInput").ap()
    out = nc.dram_tensor("out", (BPC, N, NF), F32, kind="ExternalOutput").ap()

    with (
        tc.tile_pool(name="consts", bufs=1) as consts,
        tc.tile_pool(name="loads", bufs=4) as loads,
        tc.tile_pool(name="big", bufs=2) as big,
        tc.tile_pool(name="work", bufs=4) as work,
        tc.tile_pool(name="perb", bufs=2) as perb,
        tc.tile_pool(name="ppool", bufs=2, space="PSUM") as ppool,
    ):
        identity = consts.tile([128, 128], F32)
        make_identity(nc, identity)
        identity_bf = consts.tile([128, 128], BF16)
        make_identity(nc, identity_bf)
        ones_f32 = consts.tile([1, N], F32)
        nc.vector.memset(ones_f32, 1.0)

        # Preload the tanh table set (~2.7us) while the first DMAs are in flight.
        warm_tanh = consts.tile([1, 8], F32)
        nc.scalar.activation(warm_tanh, ones_f32[:, :8], mybir.ActivationFunctionType.Tanh)

        # W_df in bf16 for mm1 (bias handled in the gate).
        w_df_f = consts.tile([ND, NH], F32)
        nc.sync.dma_start(w_df_f, w_df)
        w_df_bf = consts.tile([ND, NH], BF16)
        nc.vector.tensor_copy(w_df_bf, w_df_f)

        # W_fc stacked twice vertically so the partition-hi mm2 has its
        # stationary at the same base partition as its rhs.
        w_fc_f = consts.tile([2 * NH, NF], F32)
        nc.sync.dma_start(w_fc_f[:NH], w_fc)
        nc.sync.dma_start(w_fc_f[NH:], w_fc)
        w_fc_bf = consts.tile([2 * NH, NF], BF16)
        nc.vector.tensor_copy(w_fc_bf, w_fc_f)

        # W_cf plain; b_cf as a row for the K=1 bias fold.
        w_cf_sb = consts.tile([NF, NH], F32)
        nc.sync.dma_start(w_cf_sb, w_cf)
        b_cf_sb = consts.tile([1, NH], F32)
        nc.sync.dma_start(b_cf_sb, b_cf)

        # b_df as a per-partition column, duplicated for both halves.
        b_df_col_f = consts.tile([2 * NH, 1], F32)
        nc.sync.dma_start(b_df_col_f[:NH], b_df.rearrange("a h -> h a"))
        nc.sync.dma_start(b_df_col_f[NH:], b_df.rearrange("a h -> h a"))

        # --- per-molecule prep.  Partition half u of the main pipeline handles
        # j-quad 8t+4u+j', so atom_hGB[u*NH+h, 4t+j'] = atom_h[8t+4u+j', h].
        atom_hGBs = []
        for b in range(BPC):
            atom_in = loads.tile([N, NF], F32, tag="atom_in")
            nc.sync.dma_start(atom_in, atom[b])
            atomT_ps = ppool.tile([NF, N], F32, tag="out2")
            nc.tensor.transpose(atomT_ps, atom_in, identity)
            atomT = work.tile([NF, N], F32, tag="atomT")
            nc.vector.tensor_copy(atomT, atomT_ps)
            ah_ps = ppool.tile([2 * NH, N // 2], F32, tag="out2")
            atomT_q = atomT.rearrange("f (t j) -> f t j", j=8)
            for u in range(2):
                nc.tensor.matmul(
                    ah_ps[ds(u * NH, NH)],
                    w_cf_sb,
                    atomT_q[:, :, ds(4 * u, 4)],
                    start=True,
                    stop=False,
                )
                nc.tensor.matmul(
                    ah_ps[ds(u * NH, NH)],
                    b_cf_sb,
                    ones_f32[:, : N // 2],
                    start=False,
                    stop=True,
                )
            atom_hGB = perb.tile([2 * NH, N // 2], BF16, tag="atom_hGB")
            nc.vector.tensor_copy(atom_hGB, ah_ps)
            atom_hGBs.append(atom_hGB)

        distTs = [
            big.tile([ND, N * N], BF16, tag="distT", name=f"distT{b}")
            for b in range(BPC)
        ]
        tanh_fulls = [
            big.tile([2 * NF, N * N // 2], BF16, tag="tanh_full", name=f"tanhf{b}")
            for b in range(BPC)
        ]
        th2s = [
            big.tile([2 * NF, N * N // 4], F32, tag="th2", name=f"th2{b}")
            for b in range(BPC)
        ]

        def front_chunk(b, c):
            # load JC j's for all i in natural layout, casting on the wire
            dist_bf = loads.tile([N, JC * ND], BF16, tag="dist_bf")
            nc.gpsimd.dma_start(
                dist_bf, dist[b, :, ds(c * JC, JC), :].rearrange("i j d -> i (j d)")
            )
            for kk in range(JC // JB):
                tp_ps = ppool.tile([ND, JB * N], BF16, tag="tp")
                for q in range(JB):
                    nc.tensor.transpose(
                        tp_ps[:, ds(q * N, N)],
                        dist_bf[:, ds((kk * JB + q) * ND, ND)],
                        identity_bf,
                    )
                j0 = c * JC + kk * JB
                # split the PSUM->SBUF copies across vector and scalar
                dst = distTs[b][:, ds(j0 * N, JB * N)]
                if (c * (JC // JB) + kk) % 2 == 0:
                    nc.vector.tensor_copy(dst, tp_ps)
                else:
                    nc.scalar.copy(dst, tp_ps)

        def main_group_pair(b, tp):
            # two groups (2*tp, 2*tp+1); group t covers j in [8t, 8t+8), with
            # j-quads 8t..8t+3 / 8t+4..8t+7 on the partition halves.
            distT = distTs[b]
            atom_hGB = atom_hGBs[b]
            out2_ps = ppool.tile([2 * NF, 2 * 4 * N], F32, tag="out2")
            for g in range(2):
                t = 2 * tp + g
                out1_ps = ppool.tile([2 * NH, 4 * N], F32, tag="out1")
                for u in range(2):
                    nc.tensor.matmul(
                        out1_ps[ds(u * NH, NH)],
                        w_df_bf,
                        distT[:, ds((2 * t + u) * 4 * N, 4 * N)],
                        start=True,
                        stop=True,
                    )
                gatedT = work.tile([2 * NH, 4 * N], BF16, tag="gatedT")
                nc.vector.scalar_tensor_tensor(
                    gatedT.rearrange("h (j i) -> h j i", j=4),
                    out1_ps.rearrange("h (j i) -> h j i", j=4),
                    b_df_col_f,
                    atom_hGB[:, ds(4 * t, 4), None].to_broadcast((2 * NH, 4, N)),
                    op0=mybir.AluOpType.add,
                    op1=mybir.AluOpType.mult,
                )
                nc.tensor.matmul(
                    out2_ps[:NF, ds(g * 4 * N, 4 * N)],
                    w_fc_bf[:NH],
                    gatedT[:NH],
                    start=True,
                    stop=True,
                )
                nc.tensor.matmul(
                    out2_ps[NF:, ds(g * 4 * N, 4 * N)],
                    w_fc_bf[NH:],
                    gatedT[NH:],
                    start=True,
                    stop=True,
                )
            nc.scalar.activation(
                tanh_fulls[b][:, ds(tp * 8 * N, 8 * N)],
                out2_ps,
                mybir.ActivationFunctionType.Tanh,
            )

        def reduce_mol(b):
            tf = tanh_fulls[b]
            th2 = th2s[b]
            half = N * N // 4
            # stage 1 on gpsimd: fold the two j-halves (SBUF->SBUF)
            nc.gpsimd.tensor_tensor(
                th2, tf[:, :half], tf[:, half:], mybir.AluOpType.add
            )
            # stage 2 on vector: contiguous pairwise-add tree over the 32
            # remaining 128-col blocks
            res = perb.tile([2 * NF, N], F32, tag="res")
            widths = [2048, 1024, 512, 256, 128]
            srcs = th2
            for w in widths:
                dst = res if w == 128 else perb.tile([2 * NF, w], F32, tag=f"tr{w}")
                nc.vector.tensor_add(dst, srcs[:, :w], srcs[:, w : 2 * w])
                srcs = dst
            # finalize: out[b] = res^T (both partition halves summed) + atom[b]
            atom_nat = loads.tile([N, NF], F32, tag="atom_nat")
            nc.sync.dma_start(atom_nat, atom[b])
            acc = None
            for u in range(2):
                resT_ps = ppool.tile([N, NF], F32, tag="out2")
                nc.tensor.transpose(
                    resT_ps,
                    res[ds(u * NF, NF)],
                    identity[ds(u * NF, NF), ds(u * NF, NF)],
                )
                nxt = work.tile([N, NF], F32, tag=f"fin{u}")
                nc.vector.tensor_add(nxt, resT_ps, atom_nat if u == 0 else acc)
                acc = nxt
            nc.sync.dma_start(out[b], acc)

        # molecule 0 front, then molecule 0 mains interleaved with molecule 1
        # front (emission order biases the Tile scheduler's priorities).
        for c in range(N // JC):
            front_chunk(0, c)
        for tp in range(N // 8 // 2):
            if tp % 2 == 0:
                front_chunk(1, tp // 2)
            main_group_pair(0, tp)
        reduce_mol(0)
        for tp in range(N // 8 // 2):
            main_group_pair(1, tp)
        reduce_mol(1)


_NC_CACHE = None


def _get_nc():
    global _NC_CACHE
    if _NC_CACHE is None:
        nc = bacc.Bacc("TRN2", target_bir_lowering=False, debug=False)
        with tile.TileContext(nc) as tc:
            _emit(tc)
        nc.compile()
        _NC_CACHE = nc
    return _NC_CACHE


def _numpy_reference(atom, dist, mask, w_cf, w_df, w_fc, b_cf, b_df):
    dist_h = np.einsum("bijd,dh->bijh", dist, w_df) + b_df
    atom_h = np.einsum("bjf,fh->bjh", atom, w_cf) + b_cf
    gated = dist_h * atom_h[:, None, :, :]
    o = np.einsum("bijh,hf->bijf", gated, w_fc)
    o = np.tanh(o * mask[..., None])
    return (o.sum(axis=2) + atom).astype(np.float32)


def run_sharded(inputs, trace=False):
    """Shard over the batch axis, run on 8 cores, gather. Returns (out, results)."""
    atom = np.ascontiguousarray(np.asarray(inputs["atom_features"], np.float32))
    dist = np.ascontiguousarray(np.asarray(inputs["distance_matrix"], np.float32))
    w_cf = np.ascontiguousarray(np.asarray(inputs["W_cf"], np.float32))
    w_df = np.ascontiguousarray(np.asarray(inputs["W_df"], np.float32))
    w_fc = np.ascontiguousarray(np.asarray(inputs["W_fc"], np.float32))
    b_cf = np.asarray(inputs["b_cf"], np.float32).reshape(1, NH)
    b_df = np.asarray(inputs["b_df"], np.float32).reshape(1, NH)

    nc = _get_nc()
    in_maps = []
    for c in range(NCORES):
        sl = slice(c * BPC, (c + 1) * BPC)
        in_maps.append(
            {
                "dist": dist[sl],
                "atom": atom[sl],
                "w_cf": w_cf,
                "w_df": w_df,
                "w_fc": w_fc,
                "b_cf": b_cf,
                "b_df": b_df,
            }
        )
    res = run_bass_kernel_spmd(nc, in_maps, core_ids=list(range(NCORES)), trace=trace)
    out = np.concatenate([res.results[c]["out"] for c in range(NCORES)], axis=0)
    return out, res


def kernel(**inputs) -> np.ndarray:
    mask = np.asarray(inputs["distance_matrix_mask"], np.float32)
    if not np.all(mask == 1.0):
        # The hardware pipeline folds the (always-ones) mask away; keep a
        # correct path for arbitrary masks.
        return _numpy_reference(
            np.asarray(inputs["atom_features"], np.float32),
            np.asarray(inputs["distance_matrix"], np.float32),
            mask,
            np.asarray(inputs["W_cf"], np.float32),
            np.asarray(inputs["W_df"], np.float32),
            np.asarray(inputs["W_fc"], np.float32),
            np.asarray(inputs["b_cf"], np.float32),
            np.asarray(inputs["b_df"], np.float32),
        )
    out, _ = run_sharded(inputs)
    return out


# revision 16
# speedup vs baseline: 24914.2653x; 1.0316x over previous
"""DTNNStep Bass kernel for Trainium2 (8 NeuronCores, data-parallel over batch).

Computes, per molecule b:
    dist_h = dist @ W_df + b_df              # [N, N, H]
    atom_h = atom @ W_cf + b_cf              # [N, H]
    gated  = dist_h * atom_h[None, :, :]     # broadcast over i
    out    = tanh((gated @ W_fc) * mask)     # mask == 1 in this benchmark
    result = out.sum(axis=1) + atom          # [N, F]

Pipeline (v2): dist is loaded in its NATURAL [i, (j d)] layout with the
f32->bf16 cast done on the DMA wire (SWDGE), giving 12.8KB contiguous
descriptors instead of the 200B gather descriptors a j-partition rearrange
load produces (which made the SDMA engines the bottleneck).  Per-j PE
transposes build a fully transposed molecule distT[d, (j i)] in SBUF; the
matmuls then consume columns in (i, j) order through strided APs so the
j-reduction is an innermost contiguous bf16 2x reduce.  b_df is folded into
the gate as a fused scalar_tensor_tensor (dist_h + b_df) * atom_h.
"""

import os
import sys

import numpy as np

for _p in ("/opt/trn_rl_repo", os.path.expanduser("~/.axon_site/_ro/trn_rl_repo")):
    if os.path.isdir(_p) and _p not in sys.path:
        sys.path.insert(0, _p)

import concourse.bass as bass
import concourse.tile as tile
from concourse import bacc, mybir
from concourse.bass import ds
from concourse.bass_utils import run_bass_kernel_spmd
from concourse.masks import make_identity

B, N, NF, ND, NH = 16, 128, 64, 100, 64
NCORES = 8
BPC = B // NCORES  # molecules per core

F32 = mybir.dt.float32
BF16 = mybir.dt.bfloat16

JC = 16  # j's per dist DMA chunk (small: first chunk must land early)
JB = 8  # j's per transpose batch (one PSUM bank)


def _emit(tc):
    nc = tc.nc
    dist = nc.dram_tensor("dist", (BPC, N, N, ND), F32, kind="ExternalInput").ap()
    atom = nc.dram_tensor("atom", (BPC, N, NF), F32, kind="ExternalInput").ap()
    w_cf = nc.dram_tensor("w_cf", (NF, NH), F32, kind="ExternalInput").ap()
    w_df = nc.dram_tensor("w_df", (ND, NH), F32, kind="ExternalInput").ap()
    w_fc = nc.dram_tensor("w_fc", (NH, NF), F32, kind="ExternalInput").ap()
    b_cf = nc.dram_tensor("b_cf", (1, NH), F32, kind="ExternalInput").ap()
    b_df = nc.dram_tensor("b_df", (1, NH), F32, kind="ExternalInput").ap()
    out = nc.dram_tensor("out", (BPC, N, NF), F32, kind="ExternalOutput").ap()

    with (
        tc.tile_pool(name="consts", bufs=1) as consts,
        tc.tile_pool(name="loads", bufs=2) as loads,
        tc.tile_pool(name="big", bufs=2) as big,
        tc.tile_pool(name="work", bufs=4) as work,
        tc.tile_pool(name="perb", bufs=2) as perb,
        tc.tile_pool(name="ppool", bufs=2, space="PSUM") as ppool,
    ):
        identity = consts.tile([128, 128], F32)
        make_identity(nc, identity)
        identity_bf = consts.tile([128, 128], BF16)
        make_identity(nc, identity_bf)
        ones_f32 = consts.tile([1, N], F32)
        nc.vector.memset(ones_f32, 1.0)

        # Preload the tanh table set (~2.7us) while the first DMAs are in flight.
        warm_tanh = consts.tile([1, 8], F32)
        nc.scalar.activation(warm_tanh, ones_f32[:, :8], mybir.ActivationFunctionType.Tanh)

        # W_df in bf16 for mm1 (bias handled in the gate).
        w_df_f = consts.tile([ND, NH], F32)
        nc.sync.dma_start(w_df_f, w_df)
        w_df_bf = consts.tile([ND, NH], BF16)
        nc.vector.tensor_copy(w_df_bf, w_df_f)

        # W_fc stacked twice vertically so the partition-hi mm2 has its
        # stationary at the same base partition as its rhs.
        w_fc_f = consts.tile([2 * NH, NF], F32)
        nc.sync.dma_start(w_fc_f[:NH], w_fc)
        nc.sync.dma_start(w_fc_f[NH:], w_fc)
        w_fc_bf = consts.tile([2 * NH, NF], BF16)
        nc.vector.tensor_copy(w_fc_bf, w_fc_f)

        # W_cf plain; b_cf as a row for the K=1 bias fold.
        w_cf_sb = consts.tile([NF, NH], F32)
        nc.sync.dma_start(w_cf_sb, w_cf)
        b_cf_sb = consts.tile([1, NH], F32)
        nc.sync.dma_start(b_cf_sb, b_cf)

        # b_df as a per-partition column, duplicated for both halves.
        b_df_col_f = consts.tile([2 * NH, 1], F32)
        nc.sync.dma_start(b_df_col_f[:NH], b_df.rearrange("a h -> h a"))
        nc.sync.dma_start(b_df_col_f[NH:], b_df.rearrange("a h -> h a"))

        # --- per-molecule prep.  Partition half u of the main pipeline handles
        # j-quad 8t+4u+j', so atom_hGB[u*NH+h, 4t+j'] = atom_h[8t+4u+j', h].
        atom_hGBs = []
        for b in range(BPC):
            atom_in = loads.tile([N, NF], F32, tag="atom_in")
            nc.sync.dma_start(atom_in, atom[b])
            atomT_ps = ppool.tile([NF, N], F32, tag="out2")
            nc.tensor.transpose(atomT_ps, atom_in, identity)
            atomT = work.tile([NF, N], F32, tag="atomT")
            nc.vector.tensor_copy(atomT, atomT_ps)
            ah_ps = ppool.tile([2 * NH, N // 2], F32, tag="out2")
            atomT_q = atomT.rearrange("f (t j) -> f t j", j=8)
            for u in range(2):
                nc.tensor.matmul(
                    ah_ps[ds(u * NH, NH)],
                    w_cf_sb,
                    atomT_q[:, :, ds(4 * u, 4)],
                    start=True,
                    stop=False,
                )
                nc.tensor.matmul(
                    ah_ps[ds(u * NH, NH)],
                    b_cf_sb,
                    ones_f32[:, : N // 2],
                    start=False,
                    stop=True,
                )
            atom_hGB = perb.tile([2 * NH, N // 2], BF16, tag="atom_hGB")
            nc.vector.tensor_copy(atom_hGB, ah_ps)
            atom_hGBs.append(atom_hGB)

        distTs = [
            big.tile([ND, N * N], BF16, tag="distT", name=f"distT{b}")
            for b in range(BPC)
        ]
        tanh_fulls = [
            big.tile([2 * NF, N * N // 2], BF16, tag="tanh_full", name=f"tanhf{b}")
            for b in range(BPC)
        ]
        th2s = [
            big.tile([2 * NF, N * N // 4], F32, tag="th2", name=f"th2{b}")
            for b in range(BPC)
        ]

        def front_chunk(b, c):
            # load JC j's for all i in natural layout, casting on the wire
            dist_bf = loads.tile([N, JC * ND], BF16, tag="dist_bf")
            nc.gpsimd.dma_start(
                dist_bf, dist[b, :, ds(c * JC, JC), :].rearrange("i j d -> i (j d)")
            )
            for kk in range(JC // JB):
                tp_ps = ppool.tile([ND, JB * N], BF16, tag="tp")
                for q in range(JB):
                    nc.tensor.transpose(
                        tp_ps[:, ds(q * N, N)],
                        dist_bf[:, ds((kk * JB + q) * ND, ND)],
                        identity_bf,
                    )
                j0 = c * JC + kk * JB
                # split the PSUM->SBUF copies across vector and scalar
                dst = distTs[b][:, ds(j0 * N, JB * N)]
                if (c * (JC // JB) + kk) % 2 == 0:
                    nc.vector.tensor_copy(dst, tp_ps)
                else:
                    nc.scalar.copy(dst, tp_ps)

        def main_group_pair(b, tp):
            # two groups (2*tp, 2*tp+1); group t covers j in [8t, 8t+8), with
            # j-quads 8t..8t+3 / 8t+4..8t+7 on the partition halves.
            distT = distTs[b]
            atom_hGB = atom_hGBs[b]
            out2_ps = ppool.tile([2 * NF, 2 * 4 * N], F32, tag="out2")
            for g in range(2):
                t = 2 * tp + g
                out1_ps = ppool.tile([2 * NH, 4 * N], F32, tag="out1")
                for u in range(2):
                    nc.tensor.matmul(
                        out1_ps[ds(u * NH, NH)],
                        w_df_bf,
                        distT[:, ds((2 * t + u) * 4 * N, 4 * N)],
                        start=True,
                        stop=True,
                    )
                gatedT = work.tile([2 * NH, 4 * N], BF16, tag="gatedT")
                nc.vector.scalar_tensor_tensor(
                    gatedT.rearrange("h (j i) -> h j i", j=4),
                    out1_ps.rearrange("h (j i) -> h j i", j=4),
                    b_df_col_f,
                    atom_hGB[:, ds(4 * t, 4), None].to_broadcast((2 * NH, 4, N)),
                    op0=mybir.AluOpType.add,
                    op1=mybir.AluOpType.mult,
                )
                nc.tensor.matmul(
                    out2_ps[:NF, ds(g * 4 * N, 4 * N)],
                    w_fc_bf[:NH],
                    gatedT[:NH],
                    start=True,
                    stop=True,
                )
                nc.tensor.matmul(
                    out2_ps[NF:, ds(g * 4 * N, 4 * N)],
                    w_fc_bf[NH:],
                    gatedT[NH:],
                    start=True,
                    stop=True,
                )
            nc.scalar.activation(
                tanh_fulls[b][:, ds(tp * 8 * N, 8 * N)],
                out2_ps,
                mybir.ActivationFunctionType.Tanh,
            )

        partials = {}

        def reduce_half(b, h):
            # Fold one half-molecule (tanh of groups 8h..8h+7, cols h*4096
            # onward) down to a [2NF, N] partial: gpsimd folds 2048-col pairs,
            # vector tree-adds the rest.  Runs overlapped with later mains.
            tf = tanh_fulls[b]
            th2 = th2s[b]
            base = h * 4 * N * 8
            nc.gpsimd.tensor_tensor(
                th2[:, ds(h * 2048, 2048)],
                tf[:, ds(base, 2048)],
                tf[:, ds(base + 2048, 2048)],
                mybir.AluOpType.add,
            )
            srcs = th2[:, ds(h * 2048, 2048)]
            for w in [1024, 512, 256, 128]:
                dst = perb.tile([2 * NF, w], F32, tag=f"tr{w}", name=f"tr{w}_{b}_{h}")
                nc.vector.tensor_add(dst, srcs[:, :w], srcs[:, w : 2 * w])
                srcs = dst
            partials[(b, h)] = srcs

        def finalize_mol(b):
            res = perb.tile([2 * NF, N], F32, tag="res")
            nc.vector.tensor_add(res, partials[(b, 0)], partials[(b, 1)])
            atom_nat = loads.tile([N, NF], F32, tag="atom_nat")
            nc.sync.dma_start(atom_nat, atom[b])
            acc = None
            for u in range(2):
                resT_ps = ppool.tile([N, NF], F32, tag="out2")
                nc.tensor.transpose(
                    resT_ps,
                    res[ds(u * NF, NF)],
                    identity[ds(u * NF, NF), ds(u * NF, NF)],
                )
                nxt = work.tile([N, NF], F32, tag=f"fin{u}")
                nc.vector.tensor_add(nxt, resT_ps, atom_nat if u == 0 else acc)
                acc = nxt
            nc.sync.dma_start(out[b], acc)

        # molecule 0 front first, then molecule 0 mains interleaved with
        # molecule 1 front (emission order biases scheduler priorities).
        # Half-molecule reductions fire as soon as their tanh groups land.
        NTP = N // 8 // 2
        for c in range(N // JC):
            front_chunk(0, c)
        for tp in range(NTP):
            front_chunk(1, tp)
            main_group_pair(0, tp)
            if tp == NTP // 2 - 1:
                reduce_half(0, 0)
        reduce_half(0, 1)
        finalize_mol(0)
        for tp in range(NTP):
            main_group_pair(1, tp)
            if tp == NTP // 2 - 1:
                reduce_half(1, 0)
        reduce_half(1, 1)
        finalize_mol(1)


_NC_CACHE = None


def _get_nc():
    global _NC_CACHE
    if _NC_CACHE is None:
        nc = bacc.Bacc("TRN2", target_bir_lowering=False, debug=False)
        with tile.TileContext(nc) as tc:
            _emit(tc)
        nc.compile()
        _NC_CACHE = nc
    return _NC_CACHE


def _numpy_reference(atom, dist, mask, w_cf, w_df, w_fc, b_cf, b_df):
    dist_h = np.einsum("bijd,dh->bijh", dist, w_df) + b_df
    atom_h = np.einsum("bjf,fh->bjh", atom, w_cf) + b_cf
    gated = dist_h * atom_h[:, None, :, :]
    o = np.einsum("bijh,hf->bijf", gated, w_fc)
    o = np.tanh(o * mask[..., None])
    return (o.sum(axis=2) + atom).astype(np.float32)


def run_sharded(inputs, trace=False):
    """Shard over the batch axis, run on 8 cores, gather. Returns (out, results)."""
    atom = np.ascontiguousarray(np.asarray(inputs["atom_features"], np.float32))
    dist = np.ascontiguousarray(np.asarray(inputs["distance_matrix"], np.float32))
    w_cf = np.ascontiguousarray(np.asarray(inputs["W_cf"], np.float32))
    w_df = np.ascontiguousarray(np.asarray(inputs["W_df"], np.float32))
    w_fc = np.ascontiguousarray(np.asarray(inputs["W_fc"], np.float32))
    b_cf = np.asarray(inputs["b_cf"], np.float32).reshape(1, NH)
    b_df = np.asarray(inputs["b_df"], np.float32).reshape(1, NH)

    nc = _get_nc()
    in_maps = []
    for c in range(NCORES):
        sl = slice(c * BPC, (c + 1) * BPC)
        in_maps.append(
            {
                "dist": dist[sl],
                "atom": atom[sl],
                "w_cf": w_cf,
                "w_df": w_df,
                "w_fc": w_fc,
                "b_cf": b_cf,
                "b_df": b_df,
            }
        )
    res = run_bass_kernel_spmd(nc, in_maps, core_ids=list(range(NCORES)), trace=trace)
    out = np.concatenate([res.results[c]["out"] for c in range(NCORES)], axis=0)
    return out, res


def kernel(**inputs) -> np.ndarray:
    mask = np.asarray(inputs["distance_matrix_mask"], np.float32)
    if not np.all(mask == 1.0):
        # The hardware pipeline folds the (always-ones) mask away; keep a
        # correct path for arbitrary masks.
        return _numpy_reference(
            np.asarray(inputs["atom_features"], np.float32),
            np.asarray(inputs["distance_matrix"], np.float32),
            mask,
            np.asarray(inputs["W_cf"], np.float32),
            np.asarray(inputs["W_df"], np.float32),
            np.asarray(inputs["W_fc"], np.float32),
            np.asarray(inputs["b_cf"], np.float32),
            np.asarray(inputs["b_df"], np.float32),
        )
    out, _ = run_sharded(inputs)
    return out
